# revision 1
# baseline (speedup 1.0000x reference)
"""OHNM (online hard negative mining) MSE loss on 8 Trainium2 NeuronCores.

Reference computation (per map, maps = character & affinity):
    all_loss = (pred - target)^2            # N = 64*512*512 pixels
    pos_sum  = sum of all_loss * weight     # over pixels with target != 0
    num_pos  = count(target != 0)
    topk     = top-1000 of all_loss over pixels with target == 0
    k        = min(1000, 4*num_pos, num_neg)
    loss     = (pos_sum + sum(topk[:k])) / (num_pos + k)
Result = loss_character + loss_affinity  (f32 scalar).

Sharding: data-parallel over batch, 8 batches per core, processed as 4 merged
[128 x 4096] tiles per map. Per tile:
  ACT   : n = Relu(1 - 1.2*t)  (exact 0/1 negative mask; targets are 0 or >0.9)
          with accum_out = per-partition negative count
  GpSimd: d = pred - target
  ACT   : l = d^2 (in place)
  DVE   : negv = l*n ; lp = l - negv (in place) ; wlp = lp*w (in place)
  ACT   : Identity(wlp) accum -> per-partition positive weighted loss
  DVE   : max8(negv) -> top-8 negative losses per (partition, tile) chunk
Host gathers the 8 cores' partials and does the exact final top-k reduce over
the candidate set. Candidate coverage is exact unless some 4096-element chunk
holds >8 of the global top-1000 (verified on host; falls back to exact numpy
in that astronomically unlikely case).
"""

import sys

sys.path.insert(0, "/opt/trn_rl_repo")

import numpy as np

import concourse.bacc as bacc
import concourse.tile as tile
from concourse import mybir
from concourse.bass_utils import run_bass_kernel_spmd

B, C, H, W = 64, 2, 512, 512
N_CORES = 8
BPC = B // N_CORES  # batches per core
P = 128
F = (H * W) // P  # 2048 elements per partition per batch-map
NTM = BPC  # tiles per map per core (1 batch each)
F2 = F  # free size of a tile
K_MAX = 1000
N_PIX = B * H * W
N_MAP = N_PIX  # pixels per map

_CACHE = {}


def _build_nc():
    f32 = mybir.dt.float32
    bf16 = mybir.dt.bfloat16
    nc = bacc.Bacc()
    pred = nc.declare_dram_parameter("pred", [BPC, C, P, F], f32, isOutput=False)
    cmap = nc.declare_dram_parameter("cmap", [BPC, P, F], f32, isOutput=False)
    amap = nc.declare_dram_parameter("amap", [BPC, P, F], f32, isOutput=False)
    cw = nc.declare_dram_parameter("cw", [BPC, P, F], f32, isOutput=False)
    aw = nc.declare_dram_parameter("aw", [BPC, P, F], f32, isOutput=False)
    cand_o = nc.declare_dram_parameter("cand", [P, 2 * NTM * 8], f32, isOutput=True)
    psum_o = nc.declare_dram_parameter("psums", [P, 2 * NTM], f32, isOutput=True)
    cnt_o = nc.declare_dram_parameter("cnts", [P, 2 * NTM], f32, isOutput=True)

    with tile.TileContext(nc) as tc:
        with (
            tc.tile_pool(name="io", bufs=4) as io,
            tc.tile_pool(name="work", bufs=4) as work,
            tc.tile_pool(name="short", bufs=2) as short,
            tc.tile_pool(name="scr", bufs=1) as scr,
            tc.tile_pool(name="singles", bufs=1) as singles,
        ):
            candt = singles.tile([P, 2 * NTM * 8], f32)
            post = singles.tile([P, 2 * NTM], f32)
            cntt = singles.tile([P, 2 * NTM], f32)

            for m, (tmap, wmap, ch) in enumerate(((cmap, cw, 0), (amap, aw, 1))):
                for bi in range(NTM):
                    j = m * NTM + bi
                    p_t = io.tile([P, F2], f32, tag="p")
                    t_t = io.tile([P, F2], f32, tag="t")
                    w_t = io.tile([P, F2], f32, tag="w")
                    # w first for lead time (it is consumed last but must not
                    # stall the tail of the DVE chain); t rides SWDGE (gpsimd)
                    # to spread queue pressure
                    nc.sync.dma_start(out=w_t, in_=wmap[bi])
                    nc.sync.dma_start(out=p_t, in_=pred[bi, ch])
                    nc.gpsimd.dma_start(out=t_t, in_=tmap[bi])

                    # n = Relu(1 - 1.2*t): exactly 1 at negatives (t == 0),
                    # exactly 0 at positives (t > 0.9); accum = negative count
                    n_t = short.tile([P, F2], bf16, tag="n")
                    nc.scalar.activation(
                        out=n_t,
                        in_=t_t,
                        func=mybir.ActivationFunctionType.Relu,
                        bias=1.0,
                        scale=-1.2,
                        accum_out=cntt[:, j : j + 1],
                    )

                    # w in bf16 so the wlp multiply hits the DVE 2x mode
                    w_b = work.tile([P, F2], bf16, tag="wb")
                    nc.scalar.copy(w_b, w_t)

                    # d = pred - target (f32, short-lived), l = d^2 in bf16
                    # so every following DVE op is pure bf16 (2x-mode eligible)
                    d = short.tile([P, F2], f32, tag="d")
                    nc.gpsimd.tensor_sub(d, p_t, t_t)
                    l_b = work.tile([P, F2], bf16, tag="lb")
                    nc.scalar.square(l_b, d)

                    # negv = l * n (negative-only losses), bf16: exact 0 at
                    # positives; ~0.4% rounding on negatives is harmless (it
                    # only feeds the top-k path and a tiny residual in pos_sum)
                    negv = work.tile([P, F2], bf16, tag="negv")
                    nc.vector.tensor_mul(negv, l_b, n_t)

                    # top-8 negative losses of this chunk (issued early: it
                    # only depends on negv)
                    nc.vector.max(out=candt[:, j * 8 : (j + 1) * 8], in_=negv)

                    # lp = l - negv (exact 0 at negatives: negv == l_b there)
                    lp_b = work.tile([P, F2], bf16, tag="lpb")
                    nc.vector.tensor_sub(lp_b, l_b, negv)
                    wlp_b = short.tile([P, F2], bf16, tag="wlpb")
                    nc.vector.tensor_mul(wlp_b, lp_b, w_b)

                    # per-partition positive weighted sum via ACT accumulator
                    junk = scr.tile([P, F2], bf16, tag="junk")
                    nc.scalar.activation(
                        out=junk,
                        in_=wlp_b,
                        func=mybir.ActivationFunctionType.Identity,
                        accum_out=post[:, j : j + 1],
                    )

            nc.sync.dma_start(out=cand_o[:], in_=candt)
            nc.sync.dma_start(out=psum_o[:], in_=post)
            nc.sync.dma_start(out=cnt_o[:], in_=cntt)
    nc.compile()
    return nc


def _get_nc():
    if "nc" not in _CACHE:
        _CACHE["nc"] = _build_nc()
    return _CACHE["nc"]


def _ohnm_np(pred, target, weight):
    """Exact numpy fallback, mirrors the reference."""
    all_loss = (pred - target) ** 2
    pos_mask = target != 0
    num_pos = int(pos_mask.sum())
    num_neg = pred.size - num_pos
    pos_sum = float((all_loss * weight)[pos_mask].astype(np.float64).sum())
    neg_loss = np.where(pos_mask, -np.inf, all_loss)
    k = min(K_MAX, 4 * num_pos, num_neg)
    topk = np.sort(neg_loss.ravel())[-K_MAX:][::-1]
    neg_sum = float(topk[:k].astype(np.float64).sum())
    return np.float32((pos_sum + neg_sum) / np.float64(num_pos + k))


def _combine_map(results, m):
    """Host-side final reduce for one map from the 8 cores' partials."""
    pos_sum = 0.0
    num_neg = 0.0
    cands = []
    for r in results:
        pos_sum += float(r["psums"][:, m * NTM : (m + 1) * NTM].astype(np.float64).sum())
        num_neg += float(r["cnts"][:, m * NTM : (m + 1) * NTM].astype(np.float64).sum())
        cands.append(r["cand"][:, m * NTM * 8 : (m + 1) * NTM * 8].reshape(P, NTM, 8))
    cand = np.stack(cands)  # [cores, P, NTM, 8] descending within each chunk
    num_neg = int(round(num_neg))
    num_pos = N_MAP - num_neg
    k = min(K_MAX, 4 * num_pos, num_neg)
    flat = np.sort(cand.ravel())[::-1]
    neg_sum = float(flat[:k].astype(np.float64).sum()) if k > 0 else 0.0
    ok = True
    if k > 0:
        tau = flat[k - 1]
        # A chunk can only hide a missed top-k element if its own 8th-largest
        # (the smallest we kept) is strictly above the k-th candidate.
        chunk_min = cand[..., 7]
        ok = not bool((chunk_min > tau).any())
    loss = np.float32((pos_sum + neg_sum) / np.float64(num_pos + k))
    return loss, ok


def kernel(output, character_map, affinity_map, character_weight, affinity_weight):
    output = np.asarray(output, dtype=np.float32)
    character_map = np.asarray(character_map, dtype=np.float32)
    affinity_map = np.asarray(affinity_map, dtype=np.float32)
    character_weight = np.asarray(character_weight, dtype=np.float32)
    affinity_weight = np.asarray(affinity_weight, dtype=np.float32)

    nc = _get_nc()
    in_maps = []
    for i in range(N_CORES):
        sl = slice(i * BPC, (i + 1) * BPC)
        in_maps.append(
            {
                "pred": np.ascontiguousarray(output[sl]).reshape(BPC, C, P, F),
                "cmap": np.ascontiguousarray(character_map[sl]).reshape(BPC, P, F),
                "amap": np.ascontiguousarray(affinity_map[sl]).reshape(BPC, P, F),
                "cw": np.ascontiguousarray(character_weight[sl]).reshape(BPC, P, F),
                "aw": np.ascontiguousarray(affinity_weight[sl]).reshape(BPC, P, F),
            }
        )
    results = run_bass_kernel_spmd(nc, in_maps, list(range(N_CORES))).results

    loss_c, ok_c = _combine_map(results, 0)
    loss_a, ok_a = _combine_map(results, 1)
    if not ok_c:
        flat = output.transpose(0, 2, 3, 1).reshape(-1, C)
        loss_c = _ohnm_np(
            flat[:, 0], character_map.reshape(-1), character_weight.reshape(-1)
        )
    if not ok_a:
        flat = output.transpose(0, 2, 3, 1).reshape(-1, C)
        loss_a = _ohnm_np(
            flat[:, 1], affinity_map.reshape(-1), affinity_weight.reshape(-1)
        )
    return np.array(np.float32(loss_c) + np.float32(loss_a), dtype=np.float32)



# revision 3
# speedup vs baseline: 1.2170x; 1.2170x over previous
"""OHNM (online hard negative mining) MSE loss on 8 Trainium2 NeuronCores.

Reference computation (per map, maps = character & affinity):
    all_loss = (pred - target)^2            # N = 64*512*512 pixels
    pos_sum  = sum of all_loss * weight     # over pixels with target != 0
    num_pos  = count(target != 0)
    topk     = top-1000 of all_loss over pixels with target == 0
    k        = min(1000, 4*num_pos, num_neg)
    loss     = (pos_sum + sum(topk[:k])) / (num_pos + k)
Result = loss_character + loss_affinity  (f32 scalar).

v2 layout: all inputs are cast to bf16 on the host (the f32->bf16 cast is the
staging-dtype choice for the device kernel; it halves HBM traffic, and the
2e-2 rel tolerance leaves >10x headroom for bf16 rounding).  Data-parallel
over batch: 8 batches per core, processed as 8 merged [128 x 4096] tiles
(2 batches each; 4 tiles per map).  Per tile:
  ACT   : n = Relu(1 - 1.2*t)  (exact 0/1 negative mask; targets are 0 or >0.9)
          with accum_out = per-partition negative count
  GpSimd: d = pred - target
  ACT   : l = d^2
  DVE   : negv = l*n ; max8(negv) -> top-8 negative losses per (partition,tile)
  DVE   : lp = l - negv (exact 0 at negatives)
  DVE   : wlp = lp*w fused with accum_out -> per-partition positive weighted sum
Host gathers the 8 cores' partials and does the exact final top-k reduce over
the candidate set.  Candidate coverage is exact unless some 4096-element chunk
holds >8 of the global top-1000 (verified on host; falls back to exact numpy
in that astronomically unlikely case).
"""

import sys

sys.path.insert(0, "/opt/trn_rl_repo")

import ml_dtypes
import numpy as np

import concourse.bacc as bacc
import concourse.tile as tile
from concourse import mybir
from concourse.bass_utils import run_bass_kernel_spmd

BF16 = ml_dtypes.bfloat16

B, C, H, W = 64, 2, 512, 512
N_CORES = 8
BPC = B // N_CORES  # batches per core
P = 128
F = 4096  # free size of a merged tile (2 batches)
SPM = BPC // 2  # merged tiles (stiles) per map per core
S = C * SPM  # stiles per core
K_MAX = 1000
N_MAP = B * H * W  # pixels per map

_CACHE = {}


def _build_nc():
    f32 = mybir.dt.float32
    bf16 = mybir.dt.bfloat16
    nc = bacc.Bacc()
    pred = nc.declare_dram_parameter("pred", [S, P, F], bf16, isOutput=False)
    targ = nc.declare_dram_parameter("targ", [S, P, F], bf16, isOutput=False)
    wgt = nc.declare_dram_parameter("wgt", [S, P, F], bf16, isOutput=False)
    cand_o = nc.declare_dram_parameter("cand", [P, S * 8], f32, isOutput=True)
    psum_o = nc.declare_dram_parameter("psums", [P, S], f32, isOutput=True)
    cnt_o = nc.declare_dram_parameter("cnts", [P, S], f32, isOutput=True)

    with tile.TileContext(nc) as tc:
        with (
            tc.tile_pool(name="io", bufs=2) as io,
            tc.tile_pool(name="work", bufs=2) as work,
            tc.tile_pool(name="singles", bufs=1) as singles,
        ):
            candt = singles.tile([P, S * 8], f32)
            post = singles.tile([P, S], f32)
            cntt = singles.tile([P, S], f32)

            for s in range(S):
                p_t = io.tile([P, F], bf16, tag="p")
                t_t = io.tile([P, F], bf16, tag="t")
                w_t = io.tile([P, F], bf16, tag="w")
                nc.sync.dma_start(out=p_t, in_=pred[s])
                nc.sync.dma_start(out=t_t, in_=targ[s])
                nc.scalar.dma_start(out=w_t, in_=wgt[s])

                # n = Relu(1 - 1.2*t): exactly 1 at negatives (t == 0),
                # exactly 0 at positives (t > 0.9); accum = negative count
                n_t = work.tile([P, F], bf16, tag="n")
                nc.scalar.activation(
                    out=n_t,
                    in_=t_t,
                    func=mybir.ActivationFunctionType.Relu,
                    bias=1.0,
                    scale=-1.2,
                    accum_out=cntt[:, s : s + 1],
                )

                # d = pred - target on gpsimd (frees the DVE for the products)
                d_t = work.tile([P, F], bf16, tag="d")
                nc.gpsimd.tensor_sub(d_t, p_t, t_t)

                # l = d^2 on ACT
                l_t = work.tile([P, F], bf16, tag="l")
                nc.scalar.square(l_t, d_t)

                # negv = l * n (negative-only losses): exact 0 at positives
                negv = work.tile([P, F], bf16, tag="negv")
                nc.vector.tensor_mul(negv, l_t, n_t)

                # top-8 negative losses of this chunk
                nc.vector.max(out=candt[:, s * 8 : (s + 1) * 8], in_=negv)

                # lp = l - negv (exact 0 at negatives: negv == l there)
                lp = work.tile([P, F], bf16, tag="lp")
                nc.vector.tensor_sub(lp, l_t, negv)

                # wlp = lp * w, fused per-partition sum -> pos_sum partial
                wlp = work.tile([P, F], bf16, tag="wlp")
                nc.vector.scalar_tensor_tensor(
                    out=wlp,
                    in0=lp,
                    scalar=1.0,
                    in1=w_t,
                    op0=mybir.AluOpType.mult,
                    op1=mybir.AluOpType.mult,
                    accum_out=post[:, s : s + 1],
                )

            nc.sync.dma_start(out=cand_o[:], in_=candt)
            nc.sync.dma_start(out=psum_o[:], in_=post)
            nc.sync.dma_start(out=cnt_o[:], in_=cntt)
    nc.compile()
    return nc


def _get_nc():
    if "nc" not in _CACHE:
        _CACHE["nc"] = _build_nc()
    return _CACHE["nc"]


def _stage(x):
    """[BPC, H, W] f32 (one map's batches for one core) -> [SPM, P, F] bf16.

    Each stile merges 2 batches along the free dim: [128, 2048 | 2048]."""
    xb = x.astype(BF16).reshape(SPM, 2, P, H * W // P)
    return np.ascontiguousarray(xb.transpose(0, 2, 1, 3)).reshape(SPM, P, F)


def _make_in_maps(output, character_map, affinity_map, character_weight, affinity_weight):
    in_maps = []
    for i in range(N_CORES):
        sl = slice(i * BPC, (i + 1) * BPC)
        pb = np.concatenate([_stage(output[sl, 0]), _stage(output[sl, 1])])
        tb = np.concatenate([_stage(character_map[sl]), _stage(affinity_map[sl])])
        wb = np.concatenate([_stage(character_weight[sl]), _stage(affinity_weight[sl])])
        in_maps.append({"pred": pb, "targ": tb, "wgt": wb})
    return in_maps


def _ohnm_np(pred, target, weight):
    """Exact numpy fallback, mirrors the reference."""
    all_loss = (pred - target) ** 2
    pos_mask = target != 0
    num_pos = int(pos_mask.sum())
    num_neg = pred.size - num_pos
    pos_sum = float((all_loss * weight)[pos_mask].astype(np.float64).sum())
    neg_loss = np.where(pos_mask, -np.inf, all_loss)
    k = min(K_MAX, 4 * num_pos, num_neg)
    topk = np.sort(neg_loss.ravel())[-K_MAX:][::-1]
    neg_sum = float(topk[:k].astype(np.float64).sum())
    return np.float32((pos_sum + neg_sum) / np.float64(num_pos + k))


def _combine_map(results, m):
    """Host-side final reduce for one map from the 8 cores' partials."""
    pos_sum = 0.0
    num_neg = 0.0
    cands = []
    for r in results:
        pos_sum += float(r["psums"][:, m * SPM : (m + 1) * SPM].astype(np.float64).sum())
        num_neg += float(r["cnts"][:, m * SPM : (m + 1) * SPM].astype(np.float64).sum())
        cands.append(r["cand"][:, m * SPM * 8 : (m + 1) * SPM * 8].reshape(P, SPM, 8))
    cand = np.stack(cands)  # [cores, P, SPM, 8] descending within each chunk
    num_neg = int(round(num_neg))
    num_pos = N_MAP - num_neg
    k = min(K_MAX, 4 * num_pos, num_neg)
    flat = np.sort(cand.ravel())[::-1]
    neg_sum = float(flat[:k].astype(np.float64).sum()) if k > 0 else 0.0
    ok = True
    if k > 0:
        tau = flat[k - 1]
        # A chunk can only hide a missed top-k element if its own 8th-largest
        # (the smallest we kept) is strictly above the k-th candidate.
        chunk_min = cand[..., 7]
        ok = not bool((chunk_min > tau).any())
    loss = np.float32((pos_sum + neg_sum) / np.float64(num_pos + k))
    return loss, ok


def kernel(output, character_map, affinity_map, character_weight, affinity_weight):
    output = np.asarray(output, dtype=np.float32)
    character_map = np.asarray(character_map, dtype=np.float32)
    affinity_map = np.asarray(affinity_map, dtype=np.float32)
    character_weight = np.asarray(character_weight, dtype=np.float32)
    affinity_weight = np.asarray(affinity_weight, dtype=np.float32)

    nc = _get_nc()
    in_maps = _make_in_maps(
        output, character_map, affinity_map, character_weight, affinity_weight
    )
    results = run_bass_kernel_spmd(nc, in_maps, list(range(N_CORES))).results

    loss_c, ok_c = _combine_map(results, 0)
    loss_a, ok_a = _combine_map(results, 1)
    if not ok_c:
        flat = output.transpose(0, 2, 3, 1).reshape(-1, C)
        loss_c = _ohnm_np(
            flat[:, 0], character_map.reshape(-1), character_weight.reshape(-1)
        )
    if not ok_a:
        flat = output.transpose(0, 2, 3, 1).reshape(-1, C)
        loss_a = _ohnm_np(
            flat[:, 1], affinity_map.reshape(-1), affinity_weight.reshape(-1)
        )
    return np.array(np.float32(loss_c) + np.float32(loss_a), dtype=np.float32)
